# revision 1
# baseline (speedup 1.0000x reference)
"""Trainium2 Bass kernel for nn_CostEstimationNet (Bayesian LSTM + linear head).

Strategy (data-parallel over batch, 8 cores, 512 batch/core):
  - Host: reparameterize weights (mu + softplus(rho)*eps), fold the sigmoid
    gates into tanh via 0.5 pre-scaling (sigmoid(z) = (tanh(z/2)+1)/2), keep
    cell/hidden state doubled (C2=2c, H2=2h) so the cell update is 3 fused
    scalar_tensor_tensor ops, and pre-halve Whh / lin_w accordingly.
  - Host: pad x to [B, T, 128] bf16 with a ones-column at index 79 (bias row),
    gate order permuted to [i, f, o, g] per group.
  - Device: one DMA-xbar-transpose per time-chunk loads x^T tiles
    [128(part=i), Tc, 512(batch)]; per step and per chain (half the batch),
    2 stationary-weight matmuls (x-side, K=80) + 1 block-diagonal recurrent
    matmul (K=20) accumulate gate pre-activations in PSUM; one ACT tanh
    writes all gates into a packed [Ti Tf To Tg | c2] buffer; STT ops
    update the cell; one ACT tanh + 1 STT produce H2; PE re-transposes H2
    (ACT copies it back to SBUF) for the next step's stationary load.
    Two chains interleave so the serial per-step dependency cycle of one
    chain overlaps the other's engine work.
"""
import os
import sys

sys.path.insert(0, "/opt/trn_rl_repo")

import numpy as np
import ml_dtypes
from contextlib import ExitStack

import concourse.bass as bass
import concourse.bacc as bacc
import concourse.tile as tile
from concourse import mybir
from concourse.bass_utils import run_bass_kernel_spmd

F32 = mybir.dt.float32
BF16 = mybir.dt.bfloat16
AF = mybir.ActivationFunctionType
OP = mybir.AluOpType

B, T, IN, H = 4096, 200, 79, 10
NCORES = 8
BC = B // NCORES            # 512 batch per core
NG = BC // 128              # 4 groups of 128
G4 = 4 * H                  # 40 gate columns per group
IN_PAD = 128
TC = 50                     # timesteps per DMA chunk
NCHUNK = T // TC
NCH = 2                     # chains (independent batch halves)
GPC = NG // NCH             # groups per chain

_prog_cache = {}
LAST_RESULTS = None
LAST_IN_MAPS = None


def _softplus(v):
    return np.log1p(np.exp(-np.abs(v))) + np.maximum(v, 0.0)


def _build_program(repeat=1, chains=NCH, ablate=""):
    gpc = NG // chains
    gcols = gpc * G4               # gate cols per chain
    hcols = gpc * H                # h cols per chain
    nc = bacc.Bacc("TRN2", target_bir_lowering=False, debug=False,
                   enable_asserts=False, num_devices=NCORES)

    xp_ap = nc.dram_tensor("xp", [BC, T, IN_PAD], BF16, kind="ExternalInput").ap()
    wih_ap = nc.dram_tensor("wih", [80, G4], BF16, kind="ExternalInput").ap()
    whh_ap = nc.dram_tensor("whh", [hcols, gcols], BF16,
                            kind="ExternalInput").ap()
    id_ap = nc.dram_tensor("ident", [128, 128], BF16, kind="ExternalInput").ap()
    lw_ap = nc.dram_tensor("lwbd", [hcols, gpc], BF16, kind="ExternalInput").ap()
    out_ap = nc.dram_tensor("out", [BC, 1], F32, kind="ExternalOutput").ap()

    with tile.TileContext(nc) as tc, ExitStack() as ctx:
        const = ctx.enter_context(tc.tile_pool(name="const", bufs=1))
        xpool = ctx.enter_context(tc.tile_pool(name="xpool", bufs=2))
        work = ctx.enter_context(tc.tile_pool(name="work", bufs=3))
        state = ctx.enter_context(tc.tile_pool(name="state", bufs=1))
        psg = ctx.enter_context(tc.tile_pool(name="psg", bufs=2, space="PSUM"))
        pst = ctx.enter_context(tc.tile_pool(name="pst", bufs=1, space="PSUM"))

        wih_sb = const.tile([80, G4], BF16)
        nc.sync.dma_start(out=wih_sb, in_=wih_ap)
        whh_sb = const.tile([hcols, gcols], BF16)
        nc.sync.dma_start(out=whh_sb, in_=whh_ap)
        id_sb = const.tile([128, 128], BF16)
        nc.sync.dma_start(out=id_sb, in_=id_ap)
        lw_sb = const.tile([hcols, gpc], BF16)
        nc.sync.dma_start(out=lw_sb, in_=lw_ap)

        # per-chain packed gate/cell buffer GB: per group [Ti Tf To Tg c2]
        GBW = 5 * H
        gb = [state.tile([128, gpc * GBW], F32, tag=f"gb{q}", name=f"gb{q}")
              for q in range(chains)]
        h2t = [state.tile([hcols, 128], BF16, tag=f"h2t{q}", name=f"h2t{q}")
               for q in range(chains)]
        for q in range(chains):
            nc.vector.memset(gb[q], 0.0)
            nc.vector.memset(h2t[q], 0.0)

        h2 = [None] * chains
        for _rep in range(repeat):
            for ck in range(NCHUNK):
                xt = xpool.tile([128, TC, BC], BF16, tag="xt")
                nc.sync.dma_start_transpose(
                    out=xt,
                    in_=xp_ap[:, ck * TC:(ck + 1) * TC, :].rearrange(
                        "b t i -> b (t i)"))
                for tl in range(TC):
                    for q in range(chains):
                        ps_g = psg.tile([128, gcols], F32, tag=f"psg{q}")
                        for j in range(gpc):
                            I = q * gpc + j
                            nc.tensor.matmul(
                                ps_g[:, G4 * j:G4 * j + G4],
                                lhsT=xt[0:80, tl, 128 * I:128 * I + 128],
                                rhs=wih_sb, start=(j == 0),
                                stop=("noh" in ablate and j == gpc - 1))
                        if "noh" not in ablate:
                            nc.tensor.matmul(ps_g, lhsT=h2t[q], rhs=whh_sb,
                                             start=False, stop=True)

                        g = gb[q].rearrange("p (g c) -> p g c", g=gpc)
                        psv = ps_g.rearrange("p (g c) -> p g c", g=gpc)
                        # ACT1: all 4 gate tanh -> packed cols 0..40/group
                        nc.scalar.activation(g[:, :, 0:G4], psv, AF.Tanh)
                        if "justx" in ablate:
                            continue
                        # STT12: [Q|P] = ([Ti|Tf] + 1) * [Tg|c2]
                        pq = work.tile([128, gpc * 2 * H], F32, tag=f"pq{q}")
                        pqv = pq.rearrange("p (g c) -> p g c", g=gpc)
                        eng12 = nc.gpsimd if "gps12" in ablate else nc.vector
                        eng12.scalar_tensor_tensor(
                            pqv, g[:, :, 0:2 * H], 1.0, g[:, :, 3 * H:5 * H],
                            op0=OP.add, op1=OP.mult)
                        # STT3: c2' = 0.5*P + Q
                        nc.vector.scalar_tensor_tensor(
                            g[:, :, 4 * H:5 * H], pqv[:, :, H:2 * H], 0.5,
                            pqv[:, :, 0:H], op0=OP.mult, op1=OP.add)
                        # ACT2: th = tanh(c2'/2)
                        th = work.tile([128, hcols], BF16, tag=f"th{q}")
                        thv = th.rearrange("p (g c) -> p g c", g=gpc)
                        nc.scalar.activation(thv, g[:, :, 4 * H:5 * H],
                                             AF.Tanh, scale=0.5)
                        # STT4: H2 = (To + 1) * th
                        h2[q] = work.tile([128, hcols], BF16, tag=f"h2{q}", name=f"h2{q}")
                        eng4 = nc.gpsimd if "gps4" in ablate else nc.vector
                        eng4.scalar_tensor_tensor(
                            h2[q].rearrange("p (g c) -> p g c", g=gpc),
                            g[:, :, 2 * H:3 * H], 1.0, thv,
                            op0=OP.add, op1=OP.mult)
                        if "notr" not in ablate and "noh" not in ablate:
                            ps_t = pst.tile([hcols, 128], BF16, tag=f"pst{q}")
                            nc.tensor.transpose(ps_t, h2[q], id_sb)
                            nc.scalar.copy(h2t[q], ps_t)

        # linear head
        ps_o = pst.tile([128, NG], F32, tag="pso")
        for q in range(chains):
            nc.tensor.matmul(ps_o[:, q * gpc:(q + 1) * gpc], lhsT=h2t[q],
                             rhs=lw_sb, start=(q == 0), stop=(q == chains - 1))
        o_sb = work.tile([128, NG], F32, tag="osb")
        nc.vector.tensor_copy(o_sb, ps_o)
        nc.sync.dma_start(
            out=out_ap.rearrange("(i p) o -> p (i o)", p=128), in_=o_sb)

    nc.compile()
    return nc


def _host_weights(wih_mu, wih_rho, wih_eps, whh_mu, whh_rho, whh_eps,
                  b_mu, b_rho, b_eps, lin_w, chains=NCH):
    gpc = NG // chains
    Wih = (np.asarray(wih_mu, np.float32)
           + _softplus(np.asarray(wih_rho, np.float32))
           * np.asarray(wih_eps, np.float32))
    Whh = (np.asarray(whh_mu, np.float32)
           + _softplus(np.asarray(whh_rho, np.float32))
           * np.asarray(whh_eps, np.float32))
    bb = (np.asarray(b_mu, np.float32)
          + _softplus(np.asarray(b_rho, np.float32))
          * np.asarray(b_eps, np.float32))

    # permute gates from reference order [i f g o] to [i f o g]
    perm = np.r_[0:H, H:2 * H, 3 * H:4 * H, 2 * H:3 * H]
    # tanh folding: i,f,o scaled by 0.5 (cols 0..30 after perm), g unscaled
    s = np.ones(G4, np.float32) * 0.5
    s[3 * H:4 * H] = 1.0
    Wih_p = Wih[:, perm] * s
    Whh_p = Whh[:, perm] * s * 0.5     # fed H2 = 2h
    bb_p = bb[perm] * s
    wih_aug = np.concatenate([Wih_p, bb_p[None, :]], 0)   # [80, 40]

    whh_bd = np.zeros((gpc * H, gpc * G4), np.float32)
    lw_bd = np.zeros((gpc * H, gpc), np.float32)
    lwh = np.asarray(lin_w, np.float32)[:, 0] * 0.5
    for j in range(gpc):
        whh_bd[H * j:H * j + H, G4 * j:G4 * j + G4] = Whh_p
        lw_bd[H * j:H * j + H, j] = lwh
    return (wih_aug.astype(ml_dtypes.bfloat16),
            whh_bd.astype(ml_dtypes.bfloat16),
            lw_bd.astype(ml_dtypes.bfloat16),
            np.eye(128, dtype=ml_dtypes.bfloat16))


def kernel(x, wih_mu, wih_rho, wih_eps, whh_mu, whh_rho, whh_eps,
           b_mu, b_rho, b_eps, lin_w, lin_b):
    global LAST_RESULTS, LAST_IN_MAPS
    x = np.asarray(x, np.float32)
    wih_b, whh_b, lw_b, id_b = _host_weights(
        wih_mu, wih_rho, wih_eps, whh_mu, whh_rho, whh_eps,
        b_mu, b_rho, b_eps, lin_w)

    x_pad = np.zeros((B, T, IN_PAD), ml_dtypes.bfloat16)
    x_pad[:, :, :IN] = x.astype(ml_dtypes.bfloat16)
    x_pad[:, :, IN] = 1.0

    if "prog" not in _prog_cache:
        _prog_cache["prog"] = _build_program(1)
    nc = _prog_cache["prog"]

    in_maps = [
        dict(xp=np.ascontiguousarray(x_pad[c * BC:(c + 1) * BC]),
             wih=wih_b, whh=whh_b, ident=id_b, lwbd=lw_b)
        for c in range(NCORES)
    ]
    LAST_IN_MAPS = in_maps
    res = run_bass_kernel_spmd(nc, in_maps, list(range(NCORES)), trace=False)
    LAST_RESULTS = res
    out = np.concatenate([res.results[c]["out"] for c in range(NCORES)], 0)
    return out + np.float32(np.asarray(lin_b, np.float32)[0])



# revision 15
# speedup vs baseline: 2.1907x; 2.1907x over previous
"""Trainium2 Bass kernel for nn_CostEstimationNet (Bayesian LSTM + linear head).

Strategy (data-parallel over batch, 8 cores, 512 batch/core):
  - Host: reparameterize weights (mu + softplus(rho)*eps), fold the sigmoid
    gates into tanh via 0.5 pre-scaling (sigmoid(z) = (tanh(z/2)+1)/2), keep
    cell/hidden state doubled (C2=2c, H2=2h) so the cell update is 3 fused
    scalar_tensor_tensor ops, and pre-halve Whh / lin_w accordingly.
  - Host: pad x to [B, T, 128] bf16 with a ones-column at index 79 (bias row),
    gate order permuted to [i, f, o, g] per group.
  - Device: one DMA-xbar-transpose per time-chunk loads x^T tiles
    [128(part=i), Tc, 512(batch)]; per step and per chain (half the batch),
    2 stationary-weight matmuls (x-side, K=80) + 1 block-diagonal recurrent
    matmul (K=20) accumulate gate pre-activations in PSUM; one ACT tanh
    writes all gates into a packed [Ti Tf To Tg | c2] buffer; STT ops
    update the cell; one ACT tanh + 1 STT produce H2; PE re-transposes H2
    and DVE copies it back to SBUF for the next step's stationary load
    (DVE, not ACT: the in-order ACT queue otherwise stalls the next
    step's gate tanh behind the copy -- worth ~9% end to end).
    Two chains interleave so the serial per-step dependency cycle of one
    chain overlaps the other's engine work.
"""
import os
import sys

sys.path.insert(0, "/opt/trn_rl_repo")

import numpy as np
import ml_dtypes
from contextlib import ExitStack

import concourse.bass as bass
import concourse.bacc as bacc
import concourse.tile as tile
from concourse import mybir
from concourse.bass_utils import run_bass_kernel_spmd

F32 = mybir.dt.float32
BF16 = mybir.dt.bfloat16
AF = mybir.ActivationFunctionType
OP = mybir.AluOpType

B, T, IN, H = 4096, 200, 79, 10
NCORES = 8
BC = B // NCORES            # 512 batch per core
NG = BC // 128              # 4 groups of 128
G4 = 4 * H                  # 40 gate columns per group
IN_PAD = 128
TC = 50                     # timesteps per DMA chunk
NCHUNK = T // TC
NCH = 2                     # chains (independent batch halves)
GPC = NG // NCH             # groups per chain

_prog_cache = {}
LAST_RESULTS = None
LAST_IN_MAPS = None


def _softplus(v):
    return np.log1p(np.exp(-np.abs(v))) + np.maximum(v, 0.0)


def _build_program(repeat=1, chains=NCH, ablate=""):
    gpc = NG // chains
    gcols = gpc * G4               # gate cols per chain
    hcols = gpc * H                # h cols per chain
    nc = bacc.Bacc("TRN2", target_bir_lowering=False, debug=False,
                   enable_asserts=False, num_devices=NCORES)

    xp_ap = nc.dram_tensor("xp", [BC, T, IN_PAD], BF16, kind="ExternalInput").ap()
    wih_ap = nc.dram_tensor("wih", [80, G4], BF16, kind="ExternalInput").ap()
    whh_ap = nc.dram_tensor("whh", [hcols, gcols], BF16,
                            kind="ExternalInput").ap()
    id_ap = nc.dram_tensor("ident", [128, 128], BF16, kind="ExternalInput").ap()
    lw_ap = nc.dram_tensor("lwbd", [hcols, gpc], BF16, kind="ExternalInput").ap()
    out_ap = nc.dram_tensor("out", [BC, 1], F32, kind="ExternalOutput").ap()

    with tile.TileContext(nc) as tc, ExitStack() as ctx:
        const = ctx.enter_context(tc.tile_pool(name="const", bufs=1))
        xpool = ctx.enter_context(tc.tile_pool(name="xpool", bufs=2))
        work = ctx.enter_context(tc.tile_pool(name="work", bufs=3))
        state = ctx.enter_context(tc.tile_pool(name="state", bufs=1))
        psg = ctx.enter_context(tc.tile_pool(name="psg", bufs=2, space="PSUM"))
        pst = ctx.enter_context(tc.tile_pool(name="pst", bufs=1, space="PSUM"))
        pstt = ctx.enter_context(tc.tile_pool(name="pstt", bufs=1,
                                              space="PSUM"))

        wih_sb = const.tile([80, G4], BF16)
        nc.sync.dma_start(out=wih_sb, in_=wih_ap)
        whh_sb = const.tile([hcols, gcols], BF16)
        nc.sync.dma_start(out=whh_sb, in_=whh_ap)
        id_sb = const.tile([128, 128], BF16)
        nc.sync.dma_start(out=id_sb, in_=id_ap)
        lw_sb = const.tile([hcols, gpc], BF16)
        nc.sync.dma_start(out=lw_sb, in_=lw_ap)

        # per-chain packed gate/cell buffer GB: per group [Ti Tf To Tg c2 ONE]
        GBW = 6 * H
        gb = [state.tile([128, gpc * GBW], F32, tag=f"gb{q}", name=f"gb{q}")
              for q in range(chains)]
        h2t = [state.tile([hcols, 128], BF16, tag=f"h2t{q}", name=f"h2t{q}")
               for q in range(chains)]
        for q in range(chains):
            nc.vector.memset(gb[q], 0.0)
            nc.vector.memset(
                gb[q].rearrange("p (g c) -> p g c", g=gpc)[:, :, 5 * H:6 * H],
                1.0)
            nc.vector.memset(h2t[q], 0.0)

        h2 = [None] * chains
        for _rep in range(repeat):
            for ck in range(NCHUNK):
                xt = xpool.tile([128, TC, BC], BF16, tag="xt")
                nc.sync.dma_start_transpose(
                    out=xt,
                    in_=xp_ap[:, ck * TC:(ck + 1) * TC, :].rearrange(
                        "b t i -> b (t i)"))
                for tl in range(TC):
                    def head(q):
                        ps_g = psg.tile([128, gcols], F32, tag=f"psg{q}")
                        for j in range(gpc):
                            I = q * gpc + j
                            nc.tensor.matmul(
                                ps_g[:, G4 * j:G4 * j + G4],
                                lhsT=xt[0:80, tl, 128 * I:128 * I + 128],
                                rhs=wih_sb, start=(j == 0),
                                stop=("noh" in ablate and j == gpc - 1))
                        if "noh" not in ablate:
                            nc.tensor.matmul(ps_g, lhsT=h2t[q], rhs=whh_sb,
                                             start=False, stop=True)

                        g = gb[q].rearrange("p (g c) -> p g c", g=gpc)
                        gbs = gb[q].rearrange("p (g s h) -> p g s h",
                                              g=gpc, h=H)
                        psv = ps_g.rearrange("p (g c) -> p g c", g=gpc)
                        # ACT1: all 4 gate tanh -> packed cols 0..40/group
                        nc.scalar.activation(g[:, :, 0:G4], psv, AF.Tanh)
                        return g, gbs

                    def tail(q, g, gbs):
                        # STT12: [Q|P] = ([Ti|Tf] + 1) * [Tg|c2]
                        pq = work.tile([128, gpc * 2 * H], F32, tag=f"pq{q}")
                        pqv = pq.rearrange("p (g c) -> p g c", g=gpc)
                        nc.vector.scalar_tensor_tensor(
                            pqv, g[:, :, 0:2 * H], 1.0,
                            gbs[:, :, 3:5, :], op0=OP.add, op1=OP.mult)
                        # STT3: c2' = 0.5*P + Q
                        nc.vector.scalar_tensor_tensor(
                            g[:, :, 4 * H:5 * H], pqv[:, :, H:2 * H], 0.5,
                            pqv[:, :, 0:H], op0=OP.mult, op1=OP.add)
                        # ACT2: th = tanh(c2'/2)
                        th = work.tile([128, hcols], BF16, tag=f"th{q}")
                        thv = th.rearrange("p (g c) -> p g c", g=gpc)
                        nc.scalar.activation(thv, g[:, :, 4 * H:5 * H],
                                             AF.Tanh, scale=0.5)
                        # STT4: H2 = (To + 1) * th
                        h2[q] = work.tile([128, hcols], BF16, tag=f"h2{q}",
                                          name=f"h2{q}")
                        nc.vector.scalar_tensor_tensor(
                            h2[q].rearrange("p (g c) -> p g c", g=gpc),
                            g[:, :, 2 * H:3 * H], 1.0, thv,
                            op0=OP.add, op1=OP.mult)
                        if "notr" not in ablate and "noh" not in ablate:
                            ps_t = pstt.tile([hcols, 128], BF16,
                                             tag=f"pst{q}")
                            nc.tensor.transpose(ps_t, h2[q], id_sb)
                            if "actcopy" in ablate:
                                nc.scalar.copy(h2t[q], ps_t)
                            else:
                                nc.vector.tensor_copy(h2t[q], ps_t)

                    if "ilv" in ablate:
                        gs = [head(q) for q in range(chains)]
                        if "justx" not in ablate:
                            for q in range(chains):
                                tail(q, *gs[q])
                    else:
                        for q in range(chains):
                            g_, gbs_ = head(q)
                            if "justx" not in ablate:
                                tail(q, g_, gbs_)

        # linear head
        ps_o = pst.tile([128, NG], F32, tag="pso")
        for q in range(chains):
            nc.tensor.matmul(ps_o[:, q * gpc:(q + 1) * gpc], lhsT=h2t[q],
                             rhs=lw_sb, start=(q == 0), stop=(q == chains - 1))
        o_sb = work.tile([128, NG], F32, tag="osb")
        nc.vector.tensor_copy(o_sb, ps_o)
        nc.sync.dma_start(
            out=out_ap.rearrange("(i p) o -> p (i o)", p=128), in_=o_sb)

    nc.compile()
    return nc


def _host_weights(wih_mu, wih_rho, wih_eps, whh_mu, whh_rho, whh_eps,
                  b_mu, b_rho, b_eps, lin_w, chains=NCH):
    gpc = NG // chains
    Wih = (np.asarray(wih_mu, np.float32)
           + _softplus(np.asarray(wih_rho, np.float32))
           * np.asarray(wih_eps, np.float32))
    Whh = (np.asarray(whh_mu, np.float32)
           + _softplus(np.asarray(whh_rho, np.float32))
           * np.asarray(whh_eps, np.float32))
    bb = (np.asarray(b_mu, np.float32)
          + _softplus(np.asarray(b_rho, np.float32))
          * np.asarray(b_eps, np.float32))

    # permute gates from reference order [i f g o] to [i f o g]
    perm = np.r_[0:H, H:2 * H, 3 * H:4 * H, 2 * H:3 * H]
    # tanh folding: i,f,o scaled by 0.5 (cols 0..30 after perm), g unscaled
    s = np.ones(G4, np.float32) * 0.5
    s[3 * H:4 * H] = 1.0
    Wih_p = Wih[:, perm] * s
    Whh_p = Whh[:, perm] * s * 0.5     # fed H2 = 2h
    bb_p = bb[perm] * s
    wih_aug = np.concatenate([Wih_p, bb_p[None, :]], 0)   # [80, 40]

    whh_bd = np.zeros((gpc * H, gpc * G4), np.float32)
    lw_bd = np.zeros((gpc * H, gpc), np.float32)
    lwh = np.asarray(lin_w, np.float32)[:, 0] * 0.5
    for j in range(gpc):
        whh_bd[H * j:H * j + H, G4 * j:G4 * j + G4] = Whh_p
        lw_bd[H * j:H * j + H, j] = lwh
    return (wih_aug.astype(ml_dtypes.bfloat16),
            whh_bd.astype(ml_dtypes.bfloat16),
            lw_bd.astype(ml_dtypes.bfloat16),
            np.eye(128, dtype=ml_dtypes.bfloat16))


def kernel(x, wih_mu, wih_rho, wih_eps, whh_mu, whh_rho, whh_eps,
           b_mu, b_rho, b_eps, lin_w, lin_b):
    global LAST_RESULTS, LAST_IN_MAPS
    x = np.asarray(x, np.float32)
    wih_b, whh_b, lw_b, id_b = _host_weights(
        wih_mu, wih_rho, wih_eps, whh_mu, whh_rho, whh_eps,
        b_mu, b_rho, b_eps, lin_w)

    x_pad = np.zeros((B, T, IN_PAD), ml_dtypes.bfloat16)
    x_pad[:, :, :IN] = x.astype(ml_dtypes.bfloat16)
    x_pad[:, :, IN] = 1.0

    if "prog" not in _prog_cache:
        _prog_cache["prog"] = _build_program(1)
    nc = _prog_cache["prog"]

    in_maps = [
        dict(xp=np.ascontiguousarray(x_pad[c * BC:(c + 1) * BC]),
             wih=wih_b, whh=whh_b, ident=id_b, lwbd=lw_b)
        for c in range(NCORES)
    ]
    LAST_IN_MAPS = in_maps
    res = run_bass_kernel_spmd(nc, in_maps, list(range(NCORES)), trace=False)
    LAST_RESULTS = res
    out = np.concatenate([res.results[c]["out"] for c in range(NCORES)], 0)
    return out + np.float32(np.asarray(lin_b, np.float32)[0])

